# revision 18
# baseline (speedup 1.0000x reference)
"""GAT layer (dense-adj variant) on 8 Trainium2 NeuronCores.

Row-parallel over destination nodes (R=1024 rows/core). Exact identity:
  out[i] = (S + fc_b*Zc[i] + sum_j E'[j,i] h_raw[j]) / (N + Zc[i])
with E' = E - 1 (zero on non-edges), h_raw = x@fc_w, Zc = sum_j E',
S = sum_j h[j] precomputed on host (fc_b's numerator effect is exactly
fc_b (x) Zc, applied per i-tile in phase D and skipped when fc_b == 0).

E' approximation (error ~5e-4 on the output):
  E'[j,i] = relu(exp(src_i)*exp(dst_j) - 1) * adj[i,j]
Exact for positive scores since exp(leaky(z)) = exp(z) for z>=0; drops the
negative-branch values exp(0.01 z)-1 in (-0.13, 0].  exp(src_i+dst_j) is a
rank-1 outer product, so NO activation-table pass over the NxN matrix is
needed.  Per strip [j on partitions, i on free] with q_j = exp(dst_j):
  DVE form:  a2 = p_rep*q_j   (TENSOR_SCALAR, AP scalar, ~0.4us)
             r  = (a2-1) max 0 (TENSOR_SCALAR, imm-only 4x, ~0.4us)
  ACT form:  r  = Relu(p_rep*scale=q_j + bias=-1)  (one fused pass)
  then       En = r * adjn -> fp8   (TENSOR_TENSOR, DVE ~1.3us / Pool ~2.7us)
En = -E' and hn = [-h_raw | -1] fp8, so the fp8 DoubleRow phase-C matmuls
(stationary En pair [128,2,128], moving hn pair [128,2,257], 0.5 cyc/row)
accumulate +E'h with Z riding as column 256.  adj ships as adjn = -adj^T
fp8e4 (halves adj DMA; 0/-1 exact).

dst never leaves PSUM: 4 extra 1-column matmuls per strip accumulate
dst_raw into a persistent PSUM bank; q_sb comes from per-chunk ACT Exps
reading that bank directly (no per-strip extracts).

Emission: phase A, then all of B (dense PE stream; ACT does hn copies,
chunk q, and its share of relu passes; DVE/Pool do the rest, paced by
interleaved xTj/adjn DMAs), then all of C (8 PSUM accumulator banks, no
tail split), then D.
"""

import numpy as np
import ml_dtypes

N = 8192
IN_DIM = 512
OUT_DIM = 256
NCORES = 8
R = N // NCORES  # 1024 dest rows per core
KT = IN_DIM // 128  # 4 k-tiles
JT = N // 128  # 64 j-strips
NP = JT // 2  # 32 j-pairs (DoubleRow)
IT = R // 128  # 8 i-tiles per core
HA = OUT_DIM + 1  # hn slot width (h | Z-ones)
HB = OUT_DIM + 1  # rhs_aug width (h | dst)
GC = 8  # strips per emission chunk

bf16 = ml_dtypes.bfloat16
f8 = ml_dtypes.float8_e4m3

_cache = {}

# Elementwise split, tuned from NTFF measurements (DVE TS 4x ~0.42us,
# DVE TT 1x fp8 ~1.3us, Pool TT ~2.7us, ACT pass ~1.25us):
N_RELU = 30  # strips whose relu pass runs fused on ACT
N_POOLEN = 30  # strips whose En masking TT runs on Pool


def _build(with_fcb):
    import concourse.tile as tile
    from concourse import bacc, mybir

    AF = mybir.ActivationFunctionType
    ALU = mybir.AluOpType
    f32 = mybir.dt.float32
    bft = mybir.dt.bfloat16
    fp8 = mybir.dt.float8e4
    DR = mybir.MatmulPerfMode.DoubleRow

    pool_en = set(np.linspace(0, JT - 1, N_POOLEN).astype(int).tolist())
    act_relu = set(np.linspace(1, JT - 2, N_RELU).astype(int).tolist())

    nc = bacc.Bacc("TRN2", target_bir_lowering=False, debug=False)

    adjn_d = nc.dram_tensor("adjn", [N, R], fp8, kind="ExternalInput").ap()
    xT_d = nc.dram_tensor("xT", [IN_DIM, N], bft, kind="ExternalInput").ap()
    xTi_d = nc.dram_tensor("xTi", [IN_DIM, R], bft, kind="ExternalInput").ap()
    # columns: [-fc_w (256) | w_dst (1)]
    rhs_aug_d = nc.dram_tensor("rhs_aug", [IN_DIM, HB], bft, kind="ExternalInput").ap()
    w_src_rep_d = nc.dram_tensor("w_src_rep", [IN_DIM, 128], bft, kind="ExternalInput").ap()
    src_bias_d = nc.dram_tensor("src_bias", [128, 1], f32, kind="ExternalInput").ap()
    # rows all = [S (256) | N]
    s_rep_d = nc.dram_tensor("s_rep", [128, HA], f32, kind="ExternalInput").ap()
    bdst_d = nc.dram_tensor("bdst", [128, 1], f32, kind="ExternalInput").ap()
    if with_fcb:
        # rows all = [fc_b (256) | 0]
        fcbz_d = nc.dram_tensor("fcbz", [128, HA], f32, kind="ExternalInput").ap()
    out_d = nc.dram_tensor("out", [R, OUT_DIM], f32, kind="ExternalOutput").ap()

    with tile.TileContext(nc) as tc:
        with (
            tc.tile_pool(name="const", bufs=1) as cpool,
            tc.tile_pool(name="hpool", bufs=1) as hpool,
            tc.tile_pool(name="xstream", bufs=12) as xpool,
            tc.tile_pool(name="astream", bufs=8) as apool,
            tc.tile_pool(name="work", bufs=6) as wpool,
            tc.tile_pool(name="estream", bufs=1) as epool,
            tc.tile_pool(name="opool", bufs=2) as opool,
        ):
            # ---- constants ----
            rhs_aug_sb = cpool.tile([128, KT * HB], bft)
            nc.sync.dma_start(
                rhs_aug_sb[:].rearrange("p (k n) -> p k n", k=KT),
                rhs_aug_d.rearrange("(k p) n -> p k n", p=128),
            )
            w_src_sb = cpool.tile([128, KT * 128], bft)
            nc.sync.dma_start(
                w_src_sb[:].rearrange("p (k n) -> p k n", k=KT),
                w_src_rep_d.rearrange("(k p) n -> p k n", p=128),
            )
            src_bias_sb = cpool.tile([128, 1], f32)
            nc.sync.dma_start(src_bias_sb[:], src_bias_d)
            s_rep_sb = cpool.tile([128, HA], f32)
            nc.sync.dma_start(s_rep_sb[:], s_rep_d)
            bdst_sb = cpool.tile([128, 1], f32)
            nc.sync.dma_start(bdst_sb[:], bdst_d)
            if with_fcb:
                fcbz_sb = cpool.tile([128, HA], f32)
                nc.sync.dma_start(fcbz_sb[:], fcbz_d)
            xTi_sb = cpool.tile([128, KT * R], bft)
            nc.sync.dma_start(
                xTi_sb[:].rearrange("p (k n) -> p k n", k=KT),
                xTi_d.rearrange("(k p) n -> p k n", p=128),
            )

            src_rep = cpool.tile([128, R], bft)
            p_rep = cpool.tile([128, R], bft)
            hn_sb = hpool.tile([128, JT * HA], fp8)
            # Z column of every hn slot = -1, set once (strip copies write
            # only cols 0:256 of each slot, so no overlap)
            nc.vector.memset(
                hn_sb[:].rearrange("p (j n) -> p j n", j=JT)[
                    :, :, OUT_DIM : OUT_DIM + 1
                ],
                -1.0,
            )
            q_sb = cpool.tile([128, JT], f32)
            neg1_sb = cpool.tile([128, 1], f32)
            nc.vector.memset(neg1_sb[:], -1.0)
            en_pairs = [None] * NP
            adj_pairs = [None] * NP

            ps_ab_cm = tc.tile_pool(name="ps_ab", bufs=3, space="PSUM")
            ps_d_cm = tc.tile_pool(name="ps_dst", bufs=1, space="PSUM")
            ps_ab = ps_ab_cm.__enter__()
            ps_d = ps_d_cm.__enter__()
            dst_ps = ps_d.tile([128, JT], f32, name="dst_ps")

            # ---- Phase A: src_rep[p, f] = src[i0+f] for all p; p_rep = exp ----
            for ch in range(R // 512):
                ps = ps_ab.tile([128, 512], f32, name="ps_a", tag="ps")
                for kt in range(KT):
                    nc.tensor.matmul(
                        ps[:],
                        w_src_sb[:, kt * 128 : (kt + 1) * 128],
                        xTi_sb[:, kt * R + ch * 512 : kt * R + (ch + 1) * 512],
                        start=(kt == 0),
                        stop=(kt == KT - 1),
                    )
                nc.scalar.activation(
                    src_rep[:, ch * 512 : (ch + 1) * 512], ps[:], AF.Identity,
                    bias=src_bias_sb[:],
                )
            nc.scalar.activation(p_rep[:], src_rep[:], AF.Exp)

            # ---- Phase B + elementwise ----
            for jt in range(JT):
                g = jt // 2
                xTj = xpool.tile([128, KT * 128], bft)
                nc.sync.dma_start(
                    xTj[:].rearrange("p (k n) -> p k n", k=KT),
                    xT_d[:, jt * 128 : (jt + 1) * 128].rearrange(
                        "(k p) n -> p k n", p=128
                    ),
                )
                if jt % 2 == 1:
                    adjp = apool.tile([128, 2 * R], fp8, name="adjp")
                    nc.sync.dma_start(
                        adjp[:].rearrange("p (kk i) -> p kk i", kk=2),
                        adjn_d[g * 256 : (g + 1) * 256, :].rearrange(
                            "(kk p) i -> p kk i", p=128
                        ),
                    )
                    adj_pairs[g] = adjp
                ps = ps_ab.tile([128, OUT_DIM], f32, name="ps_b", tag="ps")
                for kt in range(KT):
                    nc.tensor.matmul(
                        ps[:],
                        xTj[:, kt * 128 : (kt + 1) * 128],
                        rhs_aug_sb[:, kt * HB : kt * HB + OUT_DIM],
                        start=(kt == 0),
                        stop=(kt == KT - 1),
                    )
                # dst accumulates into its own persistent PSUM bank
                for kt in range(KT):
                    nc.tensor.matmul(
                        dst_ps[:, jt : jt + 1],
                        xTj[:, kt * 128 : (kt + 1) * 128],
                        rhs_aug_sb[:, kt * HB + OUT_DIM : (kt + 1) * HB],
                        start=(kt == 0),
                        stop=(kt == KT - 1),
                    )
                nc.scalar.activation(
                    hn_sb[:, jt * HA : jt * HA + OUT_DIM], ps[:, 0:OUT_DIM], AF.Copy,
                )

                if jt % GC == GC - 1:
                    gc0 = (jt // GC) * GC
                    nc.scalar.activation(
                        q_sb[:, gc0 : gc0 + GC], dst_ps[:, gc0 : gc0 + GC], AF.Exp,
                        bias=bdst_sb[:],
                    )
                    for s_jt in range(gc0, gc0 + GC):
                        s_g = s_jt // 2
                        kk = s_jt % 2
                        if en_pairs[s_g] is None:
                            en_pairs[s_g] = epool.tile(
                                [128, 2 * R], fp8, name=f"en{s_g}"
                            )
                        q_j = q_sb[:, s_jt : s_jt + 1]
                        adj_half = adj_pairs[s_g][:, kk * R : (kk + 1) * R]
                        en_half = en_pairs[s_g][:, kk * R : (kk + 1) * R]
                        r = wpool.tile([128, R], bft, name="r", tag="r")
                        if s_jt in act_relu:
                            nc.scalar.activation(
                                r[:], p_rep[:], AF.Relu, bias=neg1_sb[:], scale=q_j
                            )
                        else:
                            a2 = wpool.tile([128, R], bft, name="a2", tag="a2")
                            nc.vector.tensor_scalar_mul(a2[:], p_rep[:], q_j)
                            nc.vector.tensor_scalar(
                                r[:], a2[:], 1.0, 0.0, ALU.subtract, ALU.max
                            )
                        eng = nc.gpsimd if s_jt in pool_en else nc.vector
                        eng.tensor_mul(en_half, r[:], adj_half)

            # ---- Phase C: fp8 DoubleRow, all 8 i-tile accumulators ----
            ps_d_cm.__exit__(None, None, None)
            ps_ab_cm.__exit__(None, None, None)
            out_ps = {}
            with tc.tile_pool(name="ps_acc", bufs=1, space="PSUM") as ps_acc:
                for it in range(IT):
                    out_ps[it] = ps_acc.tile(
                        [128, HA], f32, name=f"acc{it}", tag=f"acc{it}"
                    )
                for g in range(NP):
                    en2 = en_pairs[g][:].rearrange("p (kk i) -> p kk i", kk=2)
                    hn2 = hn_sb[:, g * 2 * HA : (g + 1) * 2 * HA].rearrange(
                        "p (kk n) -> p kk n", kk=2
                    )
                    for it in range(IT):
                        nc.tensor.matmul(
                            out_ps[it][:],
                            en2[:, :, it * 128 : (it + 1) * 128],
                            hn2,
                            start=(g == 0),
                            stop=(g == NP - 1),
                            perf_mode=DR,
                        )

                # ---- Phase D: out = (acc + S [+ fc_b*Zc]) / (N + Zc) ----
                for it in range(IT):
                    u = opool.tile([128, HA], f32, tag="u")
                    nc.vector.tensor_add(u[:], out_ps[it][:], s_rep_sb[:])
                    if with_fcb:
                        u2 = opool.tile([128, HA], f32, tag="u2")
                        nc.vector.scalar_tensor_tensor(
                            u2[:], fcbz_sb[:], out_ps[it][:, OUT_DIM : OUT_DIM + 1],
                            u[:], ALU.mult, ALU.add,
                        )
                        u = u2
                    rz = opool.tile([128, 1], f32, tag="rz")
                    nc.vector.reciprocal(rz[:], u[:, OUT_DIM : OUT_DIM + 1])
                    o = opool.tile([128, OUT_DIM], f32, tag="o")
                    nc.vector.tensor_scalar_mul(o[:], u[:, 0:OUT_DIM], rz[:])
                    nc.sync.dma_start(out_d[it * 128 : (it + 1) * 128, :], o[:])

    nc.compile()
    return nc


def _prep_inputs(adj, x, fc_w, fc_b, attn_w, attn_b):
    x = np.asarray(x, np.float32)
    fc_w = np.asarray(fc_w, np.float32)
    fc_b = np.asarray(fc_b, np.float32)
    attn_w = np.asarray(attn_w, np.float32)
    a_src = fc_w @ attn_w[:OUT_DIM]
    a_dst = fc_w @ attn_w[OUT_DIM:]
    b_src = float(fc_b @ attn_w[:OUT_DIM]) + float(attn_b)
    b_dst = float(fc_b @ attn_w[OUT_DIM:])
    with_fcb = bool(np.any(fc_b))

    xT = np.ascontiguousarray(x.T).astype(bf16)
    # adjn[j, i] = -adj[i, j] in fp8 (0 / -1)
    adjn = (-np.asarray(adj, np.float32).T).astype(f8)
    rhs_aug = np.concatenate([-fc_w, a_dst[:, None]], axis=1).astype(bf16)
    w_src_rep = np.tile(a_src[:, None], (1, 128)).astype(bf16)
    src_bias = np.full((128, 1), b_src, np.float32)
    bdst = np.full((128, 1), b_dst, np.float32)
    S = x.sum(axis=0) @ fc_w + N * fc_b  # [256]
    s_rep = np.tile(
        np.concatenate([S, [np.float32(N)]]).astype(np.float32)[None, :], (128, 1)
    )

    in_maps = []
    for c in range(NCORES):
        m = {
            "adjn": np.ascontiguousarray(adjn[:, c * R : (c + 1) * R]),
            "xT": xT,
            "xTi": np.ascontiguousarray(xT[:, c * R : (c + 1) * R]),
            "rhs_aug": rhs_aug,
            "w_src_rep": w_src_rep,
            "src_bias": src_bias,
            "s_rep": s_rep,
            "bdst": bdst,
        }
        if with_fcb:
            m["fcbz"] = np.tile(
                np.concatenate([fc_b, [np.float32(0)]]).astype(np.float32)[None, :],
                (128, 1),
            )
        in_maps.append(m)
    return with_fcb, in_maps


def kernel(adj, x, fc_w, fc_b, attn_w, attn_b, _trace=False, _tmpdir=None):
    from concourse import bass_utils

    with_fcb, in_maps = _prep_inputs(adj, x, fc_w, fc_b, attn_w, attn_b)
    key = ("nc", with_fcb)
    if key not in _cache:
        _cache[key] = _build(with_fcb)
    nc = _cache[key]
    res = bass_utils.run_bass_kernel_spmd(
        nc,
        in_maps,
        core_ids=list(range(NCORES)),
        trace=_trace,
        **({"tmpdir": _tmpdir} if _tmpdir else {}),
    )
    out = np.concatenate([res.results[c]["out"] for c in range(NCORES)], axis=0)
    if _trace:
        _cache["last_exec_time_ns"] = res.exec_time_ns
        _cache["last_profile_json"] = res.profile_json
    return out


# revision 27
# speedup vs baseline: 1.0439x; 1.0439x over previous
"""GAT layer (dense-adj variant) on 8 Trainium2 NeuronCores.

Row-parallel over destination nodes (R=1024 rows/core). Exact identity:
  out[i] = (S + fc_b*Zc[i] + sum_j E'[j,i] h_raw[j]) / (N + Zc[i])
with E' = E - 1 (zero on non-edges), h_raw = x@fc_w, Zc = sum_j E',
S = sum_j h[j] precomputed on host (fc_b's numerator effect is exactly
fc_b (x) Zc, applied per i-tile in phase D and skipped when fc_b == 0).

E' approximation (error ~5e-4 on the output):
  E'[j,i] = relu(exp(src_i)*exp(dst_j) - 1) * adj[i,j]
Exact for positive scores since exp(leaky(z)) = exp(z) for z>=0; drops the
negative-branch values exp(0.01 z)-1 in (-0.13, 0].  exp(src_i+dst_j) is a
rank-1 outer product, so NO activation-table pass over the NxN matrix is
needed.  Per strip [j on partitions, i on free] with q_j = exp(dst_j):
  DVE form:  a2 = p_rep*q_j   (TENSOR_SCALAR, AP scalar, ~0.4us)
             r  = (a2-1) max 0 (TENSOR_SCALAR, imm-only 4x, ~0.4us)
  ACT form:  r  = Relu(p_rep*scale=q_j + bias=-1)  (one fused pass)
  then       En = r * adjn -> fp8   (TENSOR_TENSOR, DVE ~1.3us / Pool ~2.7us)
En = -E' and hn = [-h_raw | -1] fp8, so the fp8 DoubleRow phase-C matmuls
(stationary En pair [128,2,128], moving hn pair [128,2,257], 0.5 cyc/row)
accumulate +E'h with Z riding as column 256.  adj ships as adjn = -adj^T
fp8e4 (halves adj DMA; 0/-1 exact).

dst never leaves PSUM: 4 extra 1-column matmuls per strip accumulate
dst_raw into a persistent PSUM bank; q_sb comes from per-chunk ACT Exps
reading that bank directly (no per-strip extracts).

Emission: phase A, then all of B (dense PE stream; ACT does hn copies,
chunk q, and its share of relu passes; DVE/Pool do the rest, paced by
interleaved xTj/adjn DMAs), then all of C (8 PSUM accumulator banks, no
tail split), then D.
"""

import numpy as np
import ml_dtypes

N = 8192
IN_DIM = 512
OUT_DIM = 256
NCORES = 8
R = N // NCORES  # 1024 dest rows per core
KT = IN_DIM // 128  # 4 k-tiles
JT = N // 128  # 64 j-strips
NP = JT // 2  # 32 j-pairs (DoubleRow)
IT = R // 128  # 8 i-tiles per core
HA = OUT_DIM + 1  # hn slot width (h | Z-ones)
HB = OUT_DIM + 1  # rhs_aug width (h | dst)
GC = 4  # strips per q/w chunk

bf16 = ml_dtypes.bfloat16
f8 = ml_dtypes.float8_e4m3

_cache = {}

# Elementwise split, tuned from NTFF measurements (DVE 1-op r ~1.05us,
# DVE STT/TT En ~1.3-1.7us, Pool TT ~2.5us, ACT relu pass ~1.25us):
N_RELU = 44  # strips whose relu pass runs fused on ACT (q folded in)
N_POOLEN = 28  # relu strips whose En masking TT runs on Pool


def _build(with_fcb):
    import concourse.tile as tile
    from concourse import bacc, mybir

    AF = mybir.ActivationFunctionType
    ALU = mybir.AluOpType
    f32 = mybir.dt.float32
    bft = mybir.dt.bfloat16
    fp8 = mybir.dt.float8e4
    DR = mybir.MatmulPerfMode.DoubleRow

    act_relu = set(np.linspace(0, JT - 1, N_RELU).astype(int).tolist())
    relu_list = sorted(act_relu)
    pool_en = set(
        relu_list[i]
        for i in np.linspace(0, len(relu_list) - 1, N_POOLEN).astype(int)
    )

    nc = bacc.Bacc("TRN2", target_bir_lowering=False, debug=False)

    adjn_d = nc.dram_tensor("adjn", [N, R], fp8, kind="ExternalInput").ap()
    xT_d = nc.dram_tensor("xT", [IN_DIM, N], bft, kind="ExternalInput").ap()
    xTi_d = nc.dram_tensor("xTi", [IN_DIM, R], bft, kind="ExternalInput").ap()
    # columns: [-fc_w (256) | w_dst (1)]
    rhs_aug_d = nc.dram_tensor("rhs_aug", [IN_DIM, HB], bft, kind="ExternalInput").ap()
    w_src_rep_d = nc.dram_tensor("w_src_rep", [IN_DIM, 128], bft, kind="ExternalInput").ap()
    src_bias_d = nc.dram_tensor("src_bias", [128, 1], f32, kind="ExternalInput").ap()
    # rows all = [S (256) | N]
    s_rep_d = nc.dram_tensor("s_rep", [128, HA], f32, kind="ExternalInput").ap()
    bdst_d = nc.dram_tensor("bdst", [128, 1], f32, kind="ExternalInput").ap()
    nbdst_d = nc.dram_tensor("nbdst", [128, 1], f32, kind="ExternalInput").ap()
    if with_fcb:
        # rows all = [fc_b (256) | 0]
        fcbz_d = nc.dram_tensor("fcbz", [128, HA], f32, kind="ExternalInput").ap()
    out_d = nc.dram_tensor("out", [R, OUT_DIM], f32, kind="ExternalOutput").ap()

    with tile.TileContext(nc) as tc:
        with (
            tc.tile_pool(name="const", bufs=1) as cpool,
            tc.tile_pool(name="hpool", bufs=1) as hpool,
            tc.tile_pool(name="xstream", bufs=12) as xpool,
            tc.tile_pool(name="astream", bufs=8) as apool,
            tc.tile_pool(name="work", bufs=6) as wpool,
            tc.tile_pool(name="estream", bufs=1) as epool,
            tc.tile_pool(name="opool", bufs=2) as opool,
        ):
            # ---- constants ----
            rhs_aug_sb = cpool.tile([128, KT * HB], bft)
            nc.sync.dma_start(
                rhs_aug_sb[:].rearrange("p (k n) -> p k n", k=KT),
                rhs_aug_d.rearrange("(k p) n -> p k n", p=128),
            )
            w_src_sb = cpool.tile([128, KT * 128], bft)
            nc.sync.dma_start(
                w_src_sb[:].rearrange("p (k n) -> p k n", k=KT),
                w_src_rep_d.rearrange("(k p) n -> p k n", p=128),
            )
            src_bias_sb = cpool.tile([128, 1], f32)
            nc.sync.dma_start(src_bias_sb[:], src_bias_d)
            s_rep_sb = cpool.tile([128, HA], f32)
            nc.sync.dma_start(s_rep_sb[:], s_rep_d)
            bdst_sb = cpool.tile([128, 1], f32)
            nc.sync.dma_start(bdst_sb[:], bdst_d)
            nbdst_sb = cpool.tile([128, 1], f32)
            nc.sync.dma_start(nbdst_sb[:], nbdst_d)
            if with_fcb:
                fcbz_sb = cpool.tile([128, HA], f32)
                nc.sync.dma_start(fcbz_sb[:], fcbz_d)
            xTi_sb = cpool.tile([128, KT * R], bft)
            nc.sync.dma_start(
                xTi_sb[:].rearrange("p (k n) -> p k n", k=KT),
                xTi_d.rearrange("(k p) n -> p k n", p=128),
            )

            src_rep = cpool.tile([128, R], bft)
            p_rep = cpool.tile([128, R], bft)
            hn_sb = hpool.tile([128, JT * HA], fp8)
            # Z column of every hn slot = -1, set once (strip copies write
            # only cols 0:256 of each slot, so no overlap)
            nc.vector.memset(
                hn_sb[:].rearrange("p (j n) -> p j n", j=JT)[
                    :, :, OUT_DIM : OUT_DIM + 1
                ],
                -1.0,
            )
            q_sb = cpool.tile([128, JT], f32)
            w_sb = cpool.tile([128, JT], f32)
            neg1_sb = cpool.tile([128, 1], f32)
            nc.vector.memset(neg1_sb[:], -1.0)
            en_pairs = [None] * NP
            adj_pairs = [None] * NP

            ps_ab_cm = tc.tile_pool(name="ps_ab", bufs=5, space="PSUM")
            ps_d_cm = tc.tile_pool(name="ps_dst", bufs=1, space="PSUM")
            ps_ab = ps_ab_cm.__enter__()
            ps_d = ps_d_cm.__enter__()
            dst_ps = ps_d.tile([128, JT], f32, name="dst_ps")

            # ---- Phase A: src_rep[p, f] = src[i0+f] for all p; p_rep = exp ----
            for ch in range(R // 512):
                ps = ps_ab.tile([128, 512], f32, name="ps_a", tag="ps")
                for kt in range(KT):
                    nc.tensor.matmul(
                        ps[:],
                        w_src_sb[:, kt * 128 : (kt + 1) * 128],
                        xTi_sb[:, kt * R + ch * 512 : kt * R + (ch + 1) * 512],
                        start=(kt == 0),
                        stop=(kt == KT - 1),
                    )
                nc.scalar.activation(
                    src_rep[:, ch * 512 : (ch + 1) * 512], ps[:], AF.Identity,
                    bias=src_bias_sb[:],
                )
            nc.scalar.activation(p_rep[:], src_rep[:], AF.Exp)

            # ---- Phase B + elementwise ----
            for jt in range(JT):
                g = jt // 2
                xTj = xpool.tile([128, KT * 128], bft)
                nc.sync.dma_start(
                    xTj[:].rearrange("p (k n) -> p k n", k=KT),
                    xT_d[:, jt * 128 : (jt + 1) * 128].rearrange(
                        "(k p) n -> p k n", p=128
                    ),
                )
                if jt % 2 == 1:
                    adjp = apool.tile([128, 2 * R], fp8, name="adjp")
                    nc.sync.dma_start(
                        adjp[:].rearrange("p (kk i) -> p kk i", kk=2),
                        adjn_d[g * 256 : (g + 1) * 256, :].rearrange(
                            "(kk p) i -> p kk i", p=128
                        ),
                    )
                    adj_pairs[g] = adjp
                ps = ps_ab.tile([128, OUT_DIM], f32, name="ps_b", tag="ps")
                for kt in range(KT):
                    nc.tensor.matmul(
                        ps[:],
                        xTj[:, kt * 128 : (kt + 1) * 128],
                        rhs_aug_sb[:, kt * HB : kt * HB + OUT_DIM],
                        start=(kt == 0),
                        stop=(kt == KT - 1),
                    )
                # dst accumulates into its own persistent PSUM bank
                for kt in range(KT):
                    nc.tensor.matmul(
                        dst_ps[:, jt : jt + 1],
                        xTj[:, kt * 128 : (kt + 1) * 128],
                        rhs_aug_sb[:, kt * HB + OUT_DIM : (kt + 1) * HB],
                        start=(kt == 0),
                        stop=(kt == KT - 1),
                    )
                nc.scalar.activation(
                    hn_sb[:, jt * HA : jt * HA + OUT_DIM], ps[:, 0:OUT_DIM], AF.Copy,
                )

                if jt % GC == GC - 1:
                    gc0 = (jt // GC) * GC
                    nc.scalar.activation(
                        q_sb[:, gc0 : gc0 + GC], dst_ps[:, gc0 : gc0 + GC], AF.Exp,
                        bias=bdst_sb[:],
                    )
                    nc.scalar.activation(
                        w_sb[:, gc0 : gc0 + GC], dst_ps[:, gc0 : gc0 + GC], AF.Exp,
                        bias=nbdst_sb[:], scale=-1.0,
                    )
                    for s_jt in range(gc0, gc0 + GC):
                        s_g = s_jt // 2
                        kk = s_jt % 2
                        if en_pairs[s_g] is None:
                            en_pairs[s_g] = epool.tile(
                                [128, 2 * R], fp8, name=f"en{s_g}"
                            )
                        q_j = q_sb[:, s_jt : s_jt + 1]
                        adj_half = adj_pairs[s_g][:, kk * R : (kk + 1) * R]
                        en_half = en_pairs[s_g][:, kk * R : (kk + 1) * R]
                        r = wpool.tile([128, R], bft, name="r", tag="r")
                        if s_jt in act_relu:
                            # r = (p q - 1)_+ with q folded in; mask is plain TT
                            nc.scalar.activation(
                                r[:], p_rep[:], AF.Relu, bias=neg1_sb[:], scale=q_j
                            )
                            eng = nc.gpsimd if s_jt in pool_en else nc.vector
                            eng.tensor_mul(en_half, r[:], adj_half)
                        else:
                            # r = (p - w)_+; q applied inside the masking STT
                            nc.vector.tensor_scalar(
                                r[:], p_rep[:], w_sb[:, s_jt : s_jt + 1], 0.0,
                                ALU.subtract, ALU.max,
                            )
                            nc.vector.scalar_tensor_tensor(
                                en_half, r[:], q_j, adj_half, ALU.mult, ALU.mult,
                            )

            # ---- Phase C: fp8 DoubleRow, all 8 i-tile accumulators ----
            ps_d_cm.__exit__(None, None, None)
            ps_ab_cm.__exit__(None, None, None)
            out_ps = {}
            with tc.tile_pool(name="ps_acc", bufs=1, space="PSUM") as ps_acc:
                for it in range(IT):
                    out_ps[it] = ps_acc.tile(
                        [128, HA], f32, name=f"acc{it}", tag=f"acc{it}"
                    )
                for g in range(NP):
                    en2 = en_pairs[g][:].rearrange("p (kk i) -> p kk i", kk=2)
                    hn2 = hn_sb[:, g * 2 * HA : (g + 1) * 2 * HA].rearrange(
                        "p (kk n) -> p kk n", kk=2
                    )
                    for it in range(IT):
                        nc.tensor.matmul(
                            out_ps[it][:],
                            en2[:, :, it * 128 : (it + 1) * 128],
                            hn2,
                            start=(g == 0),
                            stop=(g == NP - 1),
                            perf_mode=DR,
                        )

                # ---- Phase D: out = (acc + S [+ fc_b*Zc]) / (N + Zc) ----
                for it in range(IT):
                    u = opool.tile([128, HA], f32, tag="u")
                    nc.vector.tensor_add(u[:], out_ps[it][:], s_rep_sb[:])
                    if with_fcb:
                        u2 = opool.tile([128, HA], f32, tag="u2")
                        nc.vector.scalar_tensor_tensor(
                            u2[:], fcbz_sb[:], out_ps[it][:, OUT_DIM : OUT_DIM + 1],
                            u[:], ALU.mult, ALU.add,
                        )
                        u = u2
                    rz = opool.tile([128, 1], f32, tag="rz")
                    nc.vector.reciprocal(rz[:], u[:, OUT_DIM : OUT_DIM + 1])
                    o = opool.tile([128, OUT_DIM], f32, tag="o")
                    nc.vector.tensor_scalar_mul(o[:], u[:, 0:OUT_DIM], rz[:])
                    nc.sync.dma_start(out_d[it * 128 : (it + 1) * 128, :], o[:])

    nc.compile()
    return nc


def _prep_inputs(adj, x, fc_w, fc_b, attn_w, attn_b):
    x = np.asarray(x, np.float32)
    fc_w = np.asarray(fc_w, np.float32)
    fc_b = np.asarray(fc_b, np.float32)
    attn_w = np.asarray(attn_w, np.float32)
    a_src = fc_w @ attn_w[:OUT_DIM]
    a_dst = fc_w @ attn_w[OUT_DIM:]
    b_src = float(fc_b @ attn_w[:OUT_DIM]) + float(attn_b)
    b_dst = float(fc_b @ attn_w[OUT_DIM:])
    with_fcb = bool(np.any(fc_b))

    xT = np.ascontiguousarray(x.T).astype(bf16)
    # adjn[j, i] = -adj[i, j] in fp8 (0 / -1)
    adjn = (-np.asarray(adj, np.float32).T).astype(f8)
    rhs_aug = np.concatenate([-fc_w, a_dst[:, None]], axis=1).astype(bf16)
    w_src_rep = np.tile(a_src[:, None], (1, 128)).astype(bf16)
    src_bias = np.full((128, 1), b_src, np.float32)
    bdst = np.full((128, 1), b_dst, np.float32)
    S = x.sum(axis=0) @ fc_w + N * fc_b  # [256]
    s_rep = np.tile(
        np.concatenate([S, [np.float32(N)]]).astype(np.float32)[None, :], (128, 1)
    )

    in_maps = []
    for c in range(NCORES):
        m = {
            "adjn": np.ascontiguousarray(adjn[:, c * R : (c + 1) * R]),
            "xT": xT,
            "xTi": np.ascontiguousarray(xT[:, c * R : (c + 1) * R]),
            "rhs_aug": rhs_aug,
            "w_src_rep": w_src_rep,
            "src_bias": src_bias,
            "s_rep": s_rep,
            "bdst": bdst,
            "nbdst": -bdst,
        }
        if with_fcb:
            m["fcbz"] = np.tile(
                np.concatenate([fc_b, [np.float32(0)]]).astype(np.float32)[None, :],
                (128, 1),
            )
        in_maps.append(m)
    return with_fcb, in_maps


def kernel(adj, x, fc_w, fc_b, attn_w, attn_b, _trace=False, _tmpdir=None):
    from concourse import bass_utils

    with_fcb, in_maps = _prep_inputs(adj, x, fc_w, fc_b, attn_w, attn_b)
    key = ("nc", with_fcb)
    if key not in _cache:
        _cache[key] = _build(with_fcb)
    nc = _cache[key]
    res = bass_utils.run_bass_kernel_spmd(
        nc,
        in_maps,
        core_ids=list(range(NCORES)),
        trace=_trace,
        **({"tmpdir": _tmpdir} if _tmpdir else {}),
    )
    out = np.concatenate([res.results[c]["out"] for c in range(NCORES)], axis=0)
    if _trace:
        _cache["last_exec_time_ns"] = res.exec_time_ns
        _cache["last_profile_json"] = res.profile_json
    return out


# revision 33
# speedup vs baseline: 1.0936x; 1.0476x over previous
"""GAT layer (dense-adj variant) on 8 Trainium2 NeuronCores.

Row-parallel over destination nodes (R=1024 rows/core). Exact identity:
  out[i] = (S + fc_b*Zc[i] + sum_j E'[j,i] h_raw[j]) / (N + Zc[i])
with E' = E - 1 (zero on non-edges), h_raw = x@fc_w, Zc = sum_j E',
S = sum_j h[j] precomputed on host (fc_b's numerator effect is exactly
fc_b (x) Zc, applied per i-tile in phase D and skipped when fc_b == 0).

E' approximation (error ~5e-4 on the output):
  E'[j,i] = relu(exp(src_i)*exp(dst_j) - 1) * adj[i,j]
Exact for positive scores since exp(leaky(z)) = exp(z) for z>=0; drops the
negative-branch values exp(0.01 z)-1 in (-0.13, 0].  exp(src_i+dst_j) is a
rank-1 outer product, so NO activation-table pass over the NxN matrix is
needed.  Per strip [j on partitions, i on free] with q_j = exp(dst_j):
  DVE form:  a2 = p_rep*q_j   (TENSOR_SCALAR, AP scalar, ~0.4us)
             r  = (a2-1) max 0 (TENSOR_SCALAR, imm-only 4x, ~0.4us)
  ACT form:  r  = Relu(p_rep*scale=q_j + bias=-1)  (one fused pass)
  then       En = r * adjn -> fp8   (TENSOR_TENSOR, DVE ~1.3us / Pool ~2.7us)
En = -E' and hn = [-h_raw | -1] fp8, so the fp8 DoubleRow phase-C matmuls
(stationary En pair [128,2,128], moving hn pair [128,2,257], 0.5 cyc/row)
accumulate +E'h with Z riding as column 256.  adj ships as adjn = -adj^T
fp8e4 (halves adj DMA; 0/-1 exact).

dst never leaves PSUM: 4 extra 1-column matmuls per strip accumulate
dst_raw into a persistent PSUM bank; q_sb comes from per-chunk ACT Exps
reading that bank directly (no per-strip extracts).

Emission: phase A, then all of B (dense PE stream; ACT does hn copies,
chunk q, and its share of relu passes; DVE/Pool do the rest, paced by
interleaved xTj/adjn DMAs), then all of C (8 PSUM accumulator banks, no
tail split), then D.
"""

import numpy as np
import ml_dtypes

N = 8192
IN_DIM = 512
OUT_DIM = 256
NCORES = 8
R = N // NCORES  # 1024 dest rows per core
KT = IN_DIM // 128  # 4 k-tiles
JT = N // 128  # 64 j-strips
NP = JT // 2  # 32 j-pairs (DoubleRow)
IT = R // 128  # 8 i-tiles per core
HA = OUT_DIM + 1  # hn slot width (h | Z-ones)
HB = OUT_DIM + 1  # rhs_aug width (h | dst)
GC = 8  # strips per q/w chunk

bf16 = ml_dtypes.bfloat16
f8 = ml_dtypes.float8_e4m3

_cache = {}

# Elementwise split at PAIR granularity so each adjn/En pair tile is
# touched by a single engine class (NTFF shows ~2.5x slowdowns on DVE ops
# that overlap other engines touching the same tiles).  Measured costs:
# DVE r-TS ~0.45us, DVE STT/TT En ~1.2-1.3us, Pool TT ~2.5us, ACT ~1.25us.
N_RELU_PAIRS = 17  # pairs whose relu pass runs fused on ACT (q folded in)
N_POOL_PAIRS = 14  # of those, pairs whose En masking TT runs on Pool


def _build(with_fcb):
    import concourse.tile as tile
    from concourse import bacc, mybir

    AF = mybir.ActivationFunctionType
    ALU = mybir.AluOpType
    f32 = mybir.dt.float32
    bft = mybir.dt.bfloat16
    fp8 = mybir.dt.float8e4
    DR = mybir.MatmulPerfMode.DoubleRow

    relu_pairs = set(np.linspace(0, NP - 1, N_RELU_PAIRS).astype(int).tolist())
    rp_list = sorted(relu_pairs)
    pool_pairs = set(
        rp_list[i] for i in np.linspace(0, len(rp_list) - 1, N_POOL_PAIRS).astype(int)
    )
    act_relu = set(s for s in range(JT) if s // 2 in relu_pairs)
    pool_en = set(s for s in range(JT) if s // 2 in pool_pairs)

    nc = bacc.Bacc("TRN2", target_bir_lowering=False, debug=False)

    adjn_d = nc.dram_tensor("adjn", [N, R], fp8, kind="ExternalInput").ap()
    xT_d = nc.dram_tensor("xT", [IN_DIM, N], bft, kind="ExternalInput").ap()
    xTi_d = nc.dram_tensor("xTi", [IN_DIM, R], bft, kind="ExternalInput").ap()
    # columns: [-fc_w (256) | w_dst (1)]
    rhs_aug_d = nc.dram_tensor("rhs_aug", [IN_DIM, HB], bft, kind="ExternalInput").ap()
    w_src_rep_d = nc.dram_tensor("w_src_rep", [IN_DIM, 128], bft, kind="ExternalInput").ap()
    src_bias_d = nc.dram_tensor("src_bias", [128, 1], f32, kind="ExternalInput").ap()
    # rows all = [S (256) | N]
    s_rep_d = nc.dram_tensor("s_rep", [128, HA], f32, kind="ExternalInput").ap()
    bdst_d = nc.dram_tensor("bdst", [128, 1], f32, kind="ExternalInput").ap()
    nbdst_d = nc.dram_tensor("nbdst", [128, 1], f32, kind="ExternalInput").ap()
    if with_fcb:
        # rows all = [fc_b (256) | 0]
        fcbz_d = nc.dram_tensor("fcbz", [128, HA], f32, kind="ExternalInput").ap()
    out_d = nc.dram_tensor("out", [R, OUT_DIM], f32, kind="ExternalOutput").ap()

    with tile.TileContext(nc) as tc:
        with (
            tc.tile_pool(name="const", bufs=1) as cpool,
            tc.tile_pool(name="hpool", bufs=1) as hpool,
            tc.tile_pool(name="xstream", bufs=12) as xpool,
            tc.tile_pool(name="astream", bufs=8) as apool,
            tc.tile_pool(name="work", bufs=6) as wpool,
            tc.tile_pool(name="estream", bufs=1) as epool,
            tc.tile_pool(name="opool", bufs=2) as opool,
        ):
            # ---- constants ----
            rhs_aug_sb = cpool.tile([128, KT * HB], bft)
            nc.sync.dma_start(
                rhs_aug_sb[:].rearrange("p (k n) -> p k n", k=KT),
                rhs_aug_d.rearrange("(k p) n -> p k n", p=128),
            )
            w_src_sb = cpool.tile([128, KT * 128], bft)
            nc.sync.dma_start(
                w_src_sb[:].rearrange("p (k n) -> p k n", k=KT),
                w_src_rep_d.rearrange("(k p) n -> p k n", p=128),
            )
            src_bias_sb = cpool.tile([128, 1], f32)
            nc.sync.dma_start(src_bias_sb[:], src_bias_d)
            s_rep_sb = cpool.tile([128, HA], f32)
            nc.sync.dma_start(s_rep_sb[:], s_rep_d)
            bdst_sb = cpool.tile([128, 1], f32)
            nc.sync.dma_start(bdst_sb[:], bdst_d)
            nbdst_sb = cpool.tile([128, 1], f32)
            nc.sync.dma_start(nbdst_sb[:], nbdst_d)
            if with_fcb:
                fcbz_sb = cpool.tile([128, HA], f32)
                nc.sync.dma_start(fcbz_sb[:], fcbz_d)
            xTi_sb = cpool.tile([128, KT * R], bft)
            nc.sync.dma_start(
                xTi_sb[:].rearrange("p (k n) -> p k n", k=KT),
                xTi_d.rearrange("(k p) n -> p k n", p=128),
            )

            src_rep = cpool.tile([128, R], bft)
            p_rep = cpool.tile([128, R], bft)
            p_rep_d = cpool.tile([128, R], bft)  # DVE readers' copy
            hn_sb = hpool.tile([128, JT * HA], fp8)
            # Z column of every hn slot = -1, set once (strip copies write
            # only cols 0:256 of each slot, so no overlap)
            nc.vector.memset(
                hn_sb[:].rearrange("p (j n) -> p j n", j=JT)[
                    :, :, OUT_DIM : OUT_DIM + 1
                ],
                -1.0,
            )
            q_sb = cpool.tile([128, JT], f32)
            w_sb = cpool.tile([128, JT], f32)
            neg1_sb = cpool.tile([128, 1], f32)
            nc.vector.memset(neg1_sb[:], -1.0)
            en_pairs = [None] * NP
            adj_pairs = [None] * NP

            ps_ab_cm = tc.tile_pool(name="ps_ab", bufs=5, space="PSUM")
            ps_d_cm = tc.tile_pool(name="ps_dst", bufs=1, space="PSUM")
            ps_ab = ps_ab_cm.__enter__()
            ps_d = ps_d_cm.__enter__()
            dst_ps = ps_d.tile([128, JT], f32, name="dst_ps")

            # ---- Phase A: src_rep[p, f] = src[i0+f] for all p; p_rep = exp ----
            for ch in range(R // 512):
                ps = ps_ab.tile([128, 512], f32, name="ps_a", tag="ps")
                for kt in range(KT):
                    nc.tensor.matmul(
                        ps[:],
                        w_src_sb[:, kt * 128 : (kt + 1) * 128],
                        xTi_sb[:, kt * R + ch * 512 : kt * R + (ch + 1) * 512],
                        start=(kt == 0),
                        stop=(kt == KT - 1),
                    )
                nc.scalar.activation(
                    src_rep[:, ch * 512 : (ch + 1) * 512], ps[:], AF.Identity,
                    bias=src_bias_sb[:],
                )
            nc.scalar.activation(p_rep[:], src_rep[:], AF.Exp)
            nc.vector.tensor_copy(p_rep_d[:], p_rep[:])

            # ---- Phase B + elementwise ----
            for jt in range(JT):
                g = jt // 2
                xTj = xpool.tile([128, KT * 128], bft)
                nc.sync.dma_start(
                    xTj[:].rearrange("p (k n) -> p k n", k=KT),
                    xT_d[:, jt * 128 : (jt + 1) * 128].rearrange(
                        "(k p) n -> p k n", p=128
                    ),
                )
                if jt % 2 == 1:
                    adjp = apool.tile([128, 2 * R], fp8, name="adjp")
                    nc.sync.dma_start(
                        adjp[:].rearrange("p (kk i) -> p kk i", kk=2),
                        adjn_d[g * 256 : (g + 1) * 256, :].rearrange(
                            "(kk p) i -> p kk i", p=128
                        ),
                    )
                    adj_pairs[g] = adjp
                ps = ps_ab.tile([128, OUT_DIM], f32, name="ps_b", tag="ps")
                for kt in range(KT):
                    nc.tensor.matmul(
                        ps[:],
                        xTj[:, kt * 128 : (kt + 1) * 128],
                        rhs_aug_sb[:, kt * HB : kt * HB + OUT_DIM],
                        start=(kt == 0),
                        stop=(kt == KT - 1),
                    )
                # dst accumulates into its own persistent PSUM bank
                for kt in range(KT):
                    nc.tensor.matmul(
                        dst_ps[:, jt : jt + 1],
                        xTj[:, kt * 128 : (kt + 1) * 128],
                        rhs_aug_sb[:, kt * HB + OUT_DIM : (kt + 1) * HB],
                        start=(kt == 0),
                        stop=(kt == KT - 1),
                    )
                nc.scalar.activation(
                    hn_sb[:, jt * HA : jt * HA + OUT_DIM], ps[:, 0:OUT_DIM], AF.Copy,
                )

                if jt % GC == GC - 1:
                    gc0 = (jt // GC) * GC
                    nc.scalar.activation(
                        q_sb[:, gc0 : gc0 + GC], dst_ps[:, gc0 : gc0 + GC], AF.Exp,
                        bias=bdst_sb[:],
                    )
                    nc.scalar.activation(
                        w_sb[:, gc0 : gc0 + GC], dst_ps[:, gc0 : gc0 + GC], AF.Exp,
                        bias=nbdst_sb[:], scale=-1.0,
                    )
                    for s_jt in range(gc0, gc0 + GC):
                        s_g = s_jt // 2
                        kk = s_jt % 2
                        if en_pairs[s_g] is None:
                            en_pairs[s_g] = epool.tile(
                                [128, 2 * R], fp8, name=f"en{s_g}"
                            )
                        q_j = q_sb[:, s_jt : s_jt + 1]
                        adj_half = adj_pairs[s_g][:, kk * R : (kk + 1) * R]
                        en_half = en_pairs[s_g][:, kk * R : (kk + 1) * R]
                        if s_jt in act_relu:
                            # r = (p q - 1)_+ with q folded in; mask is plain TT
                            r = wpool.tile([128, R], bft, name="ra", tag="ra")
                            nc.scalar.activation(
                                r[:], p_rep[:], AF.Relu, bias=neg1_sb[:], scale=q_j
                            )
                            eng = nc.gpsimd if s_jt in pool_en else nc.vector
                            eng.tensor_mul(en_half, r[:], adj_half)
                        else:
                            # r = (p - w)_+; q applied inside the masking STT
                            r = wpool.tile([128, R], bft, name="rd", tag="rd")
                            nc.vector.tensor_scalar(
                                r[:], p_rep_d[:], w_sb[:, s_jt : s_jt + 1], 0.0,
                                ALU.subtract, ALU.max,
                            )
                            nc.vector.scalar_tensor_tensor(
                                en_half, r[:], q_j, adj_half, ALU.mult, ALU.mult,
                            )

            # ---- Phase C: fp8 DoubleRow, all 8 i-tile accumulators ----
            ps_d_cm.__exit__(None, None, None)
            ps_ab_cm.__exit__(None, None, None)
            out_ps = {}
            with tc.tile_pool(name="ps_acc", bufs=1, space="PSUM") as ps_acc:
                for it in range(IT):
                    out_ps[it] = ps_acc.tile(
                        [128, HA], f32, name=f"acc{it}", tag=f"acc{it}"
                    )
                for g in range(NP):
                    en2 = en_pairs[g][:].rearrange("p (kk i) -> p kk i", kk=2)
                    hn2 = hn_sb[:, g * 2 * HA : (g + 1) * 2 * HA].rearrange(
                        "p (kk n) -> p kk n", kk=2
                    )
                    for it in range(IT):
                        nc.tensor.matmul(
                            out_ps[it][:],
                            en2[:, :, it * 128 : (it + 1) * 128],
                            hn2,
                            start=(g == 0),
                            stop=(g == NP - 1),
                            perf_mode=DR,
                        )

                # ---- Phase D: out = (acc + S [+ fc_b*Zc]) / (N + Zc) ----
                for it in range(IT):
                    u = opool.tile([128, HA], f32, tag="u")
                    nc.vector.tensor_add(u[:], out_ps[it][:], s_rep_sb[:])
                    if with_fcb:
                        u2 = opool.tile([128, HA], f32, tag="u2")
                        nc.vector.scalar_tensor_tensor(
                            u2[:], fcbz_sb[:], out_ps[it][:, OUT_DIM : OUT_DIM + 1],
                            u[:], ALU.mult, ALU.add,
                        )
                        u = u2
                    rz = opool.tile([128, 1], f32, tag="rz")
                    nc.vector.reciprocal(rz[:], u[:, OUT_DIM : OUT_DIM + 1])
                    o = opool.tile([128, OUT_DIM], f32, tag="o")
                    nc.vector.tensor_scalar_mul(o[:], u[:, 0:OUT_DIM], rz[:])
                    nc.sync.dma_start(out_d[it * 128 : (it + 1) * 128, :], o[:])

    nc.compile()
    return nc


def _prep_inputs(adj, x, fc_w, fc_b, attn_w, attn_b):
    x = np.asarray(x, np.float32)
    fc_w = np.asarray(fc_w, np.float32)
    fc_b = np.asarray(fc_b, np.float32)
    attn_w = np.asarray(attn_w, np.float32)
    a_src = fc_w @ attn_w[:OUT_DIM]
    a_dst = fc_w @ attn_w[OUT_DIM:]
    b_src = float(fc_b @ attn_w[:OUT_DIM]) + float(attn_b)
    b_dst = float(fc_b @ attn_w[OUT_DIM:])
    with_fcb = bool(np.any(fc_b))

    xT = np.ascontiguousarray(x.T).astype(bf16)
    # adjn[j, i] = -adj[i, j] in fp8 (0 / -1)
    adjn = (-np.asarray(adj, np.float32).T).astype(f8)
    rhs_aug = np.concatenate([-fc_w, a_dst[:, None]], axis=1).astype(bf16)
    w_src_rep = np.tile(a_src[:, None], (1, 128)).astype(bf16)
    src_bias = np.full((128, 1), b_src, np.float32)
    bdst = np.full((128, 1), b_dst, np.float32)
    S = x.sum(axis=0) @ fc_w + N * fc_b  # [256]
    s_rep = np.tile(
        np.concatenate([S, [np.float32(N)]]).astype(np.float32)[None, :], (128, 1)
    )

    in_maps = []
    for c in range(NCORES):
        m = {
            "adjn": np.ascontiguousarray(adjn[:, c * R : (c + 1) * R]),
            "xT": xT,
            "xTi": np.ascontiguousarray(xT[:, c * R : (c + 1) * R]),
            "rhs_aug": rhs_aug,
            "w_src_rep": w_src_rep,
            "src_bias": src_bias,
            "s_rep": s_rep,
            "bdst": bdst,
            "nbdst": -bdst,
        }
        if with_fcb:
            m["fcbz"] = np.tile(
                np.concatenate([fc_b, [np.float32(0)]]).astype(np.float32)[None, :],
                (128, 1),
            )
        in_maps.append(m)
    return with_fcb, in_maps


def kernel(adj, x, fc_w, fc_b, attn_w, attn_b, _trace=False, _tmpdir=None):
    from concourse import bass_utils

    with_fcb, in_maps = _prep_inputs(adj, x, fc_w, fc_b, attn_w, attn_b)
    key = ("nc", with_fcb)
    if key not in _cache:
        _cache[key] = _build(with_fcb)
    nc = _cache[key]
    res = bass_utils.run_bass_kernel_spmd(
        nc,
        in_maps,
        core_ids=list(range(NCORES)),
        trace=_trace,
        **({"tmpdir": _tmpdir} if _tmpdir else {}),
    )
    out = np.concatenate([res.results[c]["out"] for c in range(NCORES)], axis=0)
    if _trace:
        _cache["last_exec_time_ns"] = res.exec_time_ns
        _cache["last_profile_json"] = res.profile_json
    return out


# revision 35
# speedup vs baseline: 1.0962x; 1.0024x over previous
"""GAT layer (dense-adj variant) on 8 Trainium2 NeuronCores.

Row-parallel over destination nodes (R=1024 rows/core). Exact identity:
  out[i] = (S + fc_b*Zc[i] + sum_j E'[j,i] h_raw[j]) / (N + Zc[i])
with E' = E - 1 (zero on non-edges), h_raw = x@fc_w, Zc = sum_j E',
S = sum_j h[j] precomputed on host (fc_b's numerator effect is exactly
fc_b (x) Zc, applied per i-tile in phase D and skipped when fc_b == 0).

E' approximation (error ~5e-4 on the output):
  E'[j,i] = relu(exp(src_i)*exp(dst_j) - 1) * adj[i,j]
Exact for positive scores since exp(leaky(z)) = exp(z) for z>=0; drops the
negative-branch values exp(0.01 z)-1 in (-0.13, 0].  exp(src_i+dst_j) is a
rank-1 outer product, so NO activation-table pass over the NxN matrix is
needed.  Per strip [j on partitions, i on free] with q_j = exp(dst_j):
  DVE form:  a2 = p_rep*q_j   (TENSOR_SCALAR, AP scalar, ~0.4us)
             r  = (a2-1) max 0 (TENSOR_SCALAR, imm-only 4x, ~0.4us)
  ACT form:  r  = Relu(p_rep*scale=q_j + bias=-1)  (one fused pass)
  then       En = r * adjn -> fp8   (TENSOR_TENSOR, DVE ~1.3us / Pool ~2.7us)
En = -E' and hn = [-h_raw | -1] fp8, so the fp8 DoubleRow phase-C matmuls
(stationary En pair [128,2,128], moving hn pair [128,2,257], 0.5 cyc/row)
accumulate +E'h with Z riding as column 256.  adj ships as adjn = -adj^T
fp8e4 (halves adj DMA; 0/-1 exact).

dst never leaves PSUM: 4 extra 1-column matmuls per strip accumulate
dst_raw into a persistent PSUM bank; q_sb comes from per-chunk ACT Exps
reading that bank directly (no per-strip extracts).

Emission: phase A, then all of B (dense PE stream; ACT does hn copies,
chunk q, and its share of relu passes; DVE/Pool do the rest, paced by
interleaved xTj/adjn DMAs), then all of C (8 PSUM accumulator banks, no
tail split), then D.
"""

import numpy as np
import ml_dtypes

N = 8192
IN_DIM = 512
OUT_DIM = 256
NCORES = 8
R = N // NCORES  # 1024 dest rows per core
KT = IN_DIM // 128  # 4 k-tiles
JT = N // 128  # 64 j-strips
NP = JT // 2  # 32 j-pairs (DoubleRow)
IT = R // 128  # 8 i-tiles per core
HA = OUT_DIM + 1  # hn slot width (h | Z-ones)
HB = OUT_DIM + 1  # rhs_aug width (h | dst)
GC = 8  # strips per q/w chunk

bf16 = ml_dtypes.bfloat16
f8 = ml_dtypes.float8_e4m3

_cache = {}

# Elementwise split at PAIR granularity so each adjn/En pair tile is
# touched by a single engine class (NTFF shows ~2.5x slowdowns on DVE ops
# that overlap other engines touching the same tiles).  Measured costs:
# DVE r-TS ~0.45us, DVE STT/TT En ~1.2-1.3us, Pool TT ~2.5us, ACT ~1.25us.
N_RELU_PAIRS = 17  # pairs whose relu pass runs fused on ACT (q folded in)
N_POOL_PAIRS = 14  # of those, pairs whose En masking TT runs on Pool


def _build(with_fcb):
    import concourse.tile as tile
    from concourse import bacc, mybir

    AF = mybir.ActivationFunctionType
    ALU = mybir.AluOpType
    f32 = mybir.dt.float32
    bft = mybir.dt.bfloat16
    fp8 = mybir.dt.float8e4
    DR = mybir.MatmulPerfMode.DoubleRow

    relu_pairs = set(np.linspace(0, NP - 1, N_RELU_PAIRS).astype(int).tolist())
    rp_list = sorted(relu_pairs)
    pool_pairs = set(
        rp_list[i] for i in np.linspace(0, len(rp_list) - 1, N_POOL_PAIRS).astype(int)
    )
    act_relu = set(s for s in range(JT) if s // 2 in relu_pairs)
    pool_en = set(s for s in range(JT) if s // 2 in pool_pairs)

    nc = bacc.Bacc("TRN2", target_bir_lowering=False, debug=False)

    adjn_d = nc.dram_tensor("adjn", [N, R], fp8, kind="ExternalInput").ap()
    xT_d = nc.dram_tensor("xT", [IN_DIM, N], bft, kind="ExternalInput").ap()
    xTi_d = nc.dram_tensor("xTi", [IN_DIM, R], bft, kind="ExternalInput").ap()
    # columns: [-fc_w (256) | w_dst (1)]
    rhs_aug_d = nc.dram_tensor("rhs_aug", [IN_DIM, HB], bft, kind="ExternalInput").ap()
    w_src_rep_d = nc.dram_tensor("w_src_rep", [IN_DIM, 128], bft, kind="ExternalInput").ap()
    src_bias_d = nc.dram_tensor("src_bias", [128, 1], f32, kind="ExternalInput").ap()
    # rows all = [S (256) | N]
    s_rep_d = nc.dram_tensor("s_rep", [128, HA], f32, kind="ExternalInput").ap()
    bdst_d = nc.dram_tensor("bdst", [128, 1], f32, kind="ExternalInput").ap()
    nbdst_d = nc.dram_tensor("nbdst", [128, 1], f32, kind="ExternalInput").ap()
    if with_fcb:
        # rows all = [fc_b (256) | 0]
        fcbz_d = nc.dram_tensor("fcbz", [128, HA], f32, kind="ExternalInput").ap()
    out_d = nc.dram_tensor("out", [R, OUT_DIM], f32, kind="ExternalOutput").ap()

    with tile.TileContext(nc) as tc:
        with (
            tc.tile_pool(name="const", bufs=1) as cpool,
            tc.tile_pool(name="hpool", bufs=1) as hpool,
            tc.tile_pool(name="xstream", bufs=12) as xpool,
            tc.tile_pool(name="astream", bufs=8) as apool,
            tc.tile_pool(name="work", bufs=6) as wpool,
            tc.tile_pool(name="estream", bufs=1) as epool,
            tc.tile_pool(name="opool", bufs=2) as opool,
        ):
            # ---- constants (phase-A inputs first: they gate everything) ----
            w_src_sb = cpool.tile([128, KT * 128], bft)
            nc.sync.dma_start(
                w_src_sb[:].rearrange("p (k n) -> p k n", k=KT),
                w_src_rep_d.rearrange("(k p) n -> p k n", p=128),
            )
            src_bias_sb = cpool.tile([128, 1], f32)
            nc.sync.dma_start(src_bias_sb[:], src_bias_d)
            xTi_sb = cpool.tile([128, KT * R], bft)
            nc.sync.dma_start(
                xTi_sb[:].rearrange("p (k n) -> p k n", k=KT),
                xTi_d.rearrange("(k p) n -> p k n", p=128),
            )
            rhs_aug_sb = cpool.tile([128, KT * HB], bft)
            nc.sync.dma_start(
                rhs_aug_sb[:].rearrange("p (k n) -> p k n", k=KT),
                rhs_aug_d.rearrange("(k p) n -> p k n", p=128),
            )
            bdst_sb = cpool.tile([128, 1], f32)
            nc.sync.dma_start(bdst_sb[:], bdst_d)
            nbdst_sb = cpool.tile([128, 1], f32)
            nc.sync.dma_start(nbdst_sb[:], nbdst_d)
            s_rep_sb = cpool.tile([128, HA], f32)
            nc.sync.dma_start(s_rep_sb[:], s_rep_d)
            if with_fcb:
                fcbz_sb = cpool.tile([128, HA], f32)
                nc.sync.dma_start(fcbz_sb[:], fcbz_d)

            src_rep = cpool.tile([128, R], bft)
            p_rep = cpool.tile([128, R], bft)
            p_rep_d = cpool.tile([128, R], bft)  # DVE readers' copy
            hn_sb = hpool.tile([128, JT * HA], fp8)
            # Z column of every hn slot = -1, set once (strip copies write
            # only cols 0:256 of each slot, so no overlap)
            nc.vector.memset(
                hn_sb[:].rearrange("p (j n) -> p j n", j=JT)[
                    :, :, OUT_DIM : OUT_DIM + 1
                ],
                -1.0,
            )
            q_sb = cpool.tile([128, JT], f32)
            w_sb = cpool.tile([128, JT], f32)
            neg1_sb = cpool.tile([128, 1], f32)
            nc.vector.memset(neg1_sb[:], -1.0)
            en_pairs = [None] * NP
            adj_pairs = [None] * NP

            ps_ab_cm = tc.tile_pool(name="ps_ab", bufs=5, space="PSUM")
            ps_d_cm = tc.tile_pool(name="ps_dst", bufs=1, space="PSUM")
            ps_ab = ps_ab_cm.__enter__()
            ps_d = ps_d_cm.__enter__()
            dst_ps = ps_d.tile([128, JT], f32, name="dst_ps")

            # ---- Phase A: src_rep[p, f] = src[i0+f] for all p; p_rep = exp ----
            for ch in range(R // 512):
                ps = ps_ab.tile([128, 512], f32, name="ps_a", tag="ps")
                for kt in range(KT):
                    nc.tensor.matmul(
                        ps[:],
                        w_src_sb[:, kt * 128 : (kt + 1) * 128],
                        xTi_sb[:, kt * R + ch * 512 : kt * R + (ch + 1) * 512],
                        start=(kt == 0),
                        stop=(kt == KT - 1),
                    )
                nc.scalar.activation(
                    src_rep[:, ch * 512 : (ch + 1) * 512], ps[:], AF.Identity,
                    bias=src_bias_sb[:],
                )
            nc.scalar.activation(p_rep[:], src_rep[:], AF.Exp)
            nc.vector.tensor_copy(p_rep_d[:], p_rep[:])

            # ---- Phase B + elementwise ----
            for jt in range(JT):
                g = jt // 2
                xTj = xpool.tile([128, KT * 128], bft)
                nc.sync.dma_start(
                    xTj[:].rearrange("p (k n) -> p k n", k=KT),
                    xT_d[:, jt * 128 : (jt + 1) * 128].rearrange(
                        "(k p) n -> p k n", p=128
                    ),
                )
                if jt % 2 == 1:
                    adjp = apool.tile([128, 2 * R], fp8, name="adjp")
                    nc.sync.dma_start(
                        adjp[:].rearrange("p (kk i) -> p kk i", kk=2),
                        adjn_d[g * 256 : (g + 1) * 256, :].rearrange(
                            "(kk p) i -> p kk i", p=128
                        ),
                    )
                    adj_pairs[g] = adjp
                ps = ps_ab.tile([128, OUT_DIM], f32, name="ps_b", tag="ps")
                for kt in range(KT):
                    nc.tensor.matmul(
                        ps[:],
                        xTj[:, kt * 128 : (kt + 1) * 128],
                        rhs_aug_sb[:, kt * HB : kt * HB + OUT_DIM],
                        start=(kt == 0),
                        stop=(kt == KT - 1),
                    )
                # dst accumulates into its own persistent PSUM bank
                for kt in range(KT):
                    nc.tensor.matmul(
                        dst_ps[:, jt : jt + 1],
                        xTj[:, kt * 128 : (kt + 1) * 128],
                        rhs_aug_sb[:, kt * HB + OUT_DIM : (kt + 1) * HB],
                        start=(kt == 0),
                        stop=(kt == KT - 1),
                    )
                nc.scalar.activation(
                    hn_sb[:, jt * HA : jt * HA + OUT_DIM], ps[:, 0:OUT_DIM], AF.Copy,
                )

                if jt % GC == GC - 1:
                    gc0 = (jt // GC) * GC
                    nc.scalar.activation(
                        q_sb[:, gc0 : gc0 + GC], dst_ps[:, gc0 : gc0 + GC], AF.Exp,
                        bias=bdst_sb[:],
                    )
                    nc.scalar.activation(
                        w_sb[:, gc0 : gc0 + GC], dst_ps[:, gc0 : gc0 + GC], AF.Exp,
                        bias=nbdst_sb[:], scale=-1.0,
                    )
                    for s_jt in range(gc0, gc0 + GC):
                        s_g = s_jt // 2
                        kk = s_jt % 2
                        if en_pairs[s_g] is None:
                            en_pairs[s_g] = epool.tile(
                                [128, 2 * R], fp8, name=f"en{s_g}"
                            )
                        q_j = q_sb[:, s_jt : s_jt + 1]
                        adj_half = adj_pairs[s_g][:, kk * R : (kk + 1) * R]
                        en_half = en_pairs[s_g][:, kk * R : (kk + 1) * R]
                        if s_jt in act_relu:
                            # r = (p q - 1)_+ with q folded in; mask is plain TT
                            r = wpool.tile([128, R], bft, name="ra", tag="ra")
                            nc.scalar.activation(
                                r[:], p_rep[:], AF.Relu, bias=neg1_sb[:], scale=q_j
                            )
                            eng = nc.gpsimd if s_jt in pool_en else nc.vector
                            eng.tensor_mul(en_half, r[:], adj_half)
                        else:
                            # r = (p - w)_+; q applied inside the masking STT
                            r = wpool.tile([128, R], bft, name="rd", tag="rd")
                            nc.vector.tensor_scalar(
                                r[:], p_rep_d[:], w_sb[:, s_jt : s_jt + 1], 0.0,
                                ALU.subtract, ALU.max,
                            )
                            nc.vector.scalar_tensor_tensor(
                                en_half, r[:], q_j, adj_half, ALU.mult, ALU.mult,
                            )

            # ---- Phase C: fp8 DoubleRow, all 8 i-tile accumulators ----
            ps_d_cm.__exit__(None, None, None)
            ps_ab_cm.__exit__(None, None, None)
            out_ps = {}
            with tc.tile_pool(name="ps_acc", bufs=1, space="PSUM") as ps_acc:
                for it in range(IT):
                    out_ps[it] = ps_acc.tile(
                        [128, HA], f32, name=f"acc{it}", tag=f"acc{it}"
                    )
                for g in range(NP):
                    en2 = en_pairs[g][:].rearrange("p (kk i) -> p kk i", kk=2)
                    hn2 = hn_sb[:, g * 2 * HA : (g + 1) * 2 * HA].rearrange(
                        "p (kk n) -> p kk n", kk=2
                    )
                    for it in range(IT):
                        nc.tensor.matmul(
                            out_ps[it][:],
                            en2[:, :, it * 128 : (it + 1) * 128],
                            hn2,
                            start=(g == 0),
                            stop=(g == NP - 1),
                            perf_mode=DR,
                        )

                # ---- Phase D: out = (acc + S [+ fc_b*Zc]) / (N + Zc) ----
                for it in range(IT):
                    u = opool.tile([128, HA], f32, tag="u")
                    nc.vector.tensor_add(u[:], out_ps[it][:], s_rep_sb[:])
                    if with_fcb:
                        u2 = opool.tile([128, HA], f32, tag="u2")
                        nc.vector.scalar_tensor_tensor(
                            u2[:], fcbz_sb[:], out_ps[it][:, OUT_DIM : OUT_DIM + 1],
                            u[:], ALU.mult, ALU.add,
                        )
                        u = u2
                    rz = opool.tile([128, 1], f32, tag="rz")
                    nc.vector.reciprocal(rz[:], u[:, OUT_DIM : OUT_DIM + 1])
                    o = opool.tile([128, OUT_DIM], f32, tag="o")
                    nc.scalar.activation(o[:], u[:, 0:OUT_DIM], AF.Copy, scale=rz[:])
                    nc.sync.dma_start(out_d[it * 128 : (it + 1) * 128, :], o[:])

    nc.compile()
    return nc


def _prep_inputs(adj, x, fc_w, fc_b, attn_w, attn_b):
    x = np.asarray(x, np.float32)
    fc_w = np.asarray(fc_w, np.float32)
    fc_b = np.asarray(fc_b, np.float32)
    attn_w = np.asarray(attn_w, np.float32)
    a_src = fc_w @ attn_w[:OUT_DIM]
    a_dst = fc_w @ attn_w[OUT_DIM:]
    b_src = float(fc_b @ attn_w[:OUT_DIM]) + float(attn_b)
    b_dst = float(fc_b @ attn_w[OUT_DIM:])
    with_fcb = bool(np.any(fc_b))

    xT = np.ascontiguousarray(x.T).astype(bf16)
    # adjn[j, i] = -adj[i, j] in fp8 (0 / -1)
    adjn = (-np.asarray(adj, np.float32).T).astype(f8)
    rhs_aug = np.concatenate([-fc_w, a_dst[:, None]], axis=1).astype(bf16)
    w_src_rep = np.tile(a_src[:, None], (1, 128)).astype(bf16)
    src_bias = np.full((128, 1), b_src, np.float32)
    bdst = np.full((128, 1), b_dst, np.float32)
    S = x.sum(axis=0) @ fc_w + N * fc_b  # [256]
    s_rep = np.tile(
        np.concatenate([S, [np.float32(N)]]).astype(np.float32)[None, :], (128, 1)
    )

    in_maps = []
    for c in range(NCORES):
        m = {
            "adjn": np.ascontiguousarray(adjn[:, c * R : (c + 1) * R]),
            "xT": xT,
            "xTi": np.ascontiguousarray(xT[:, c * R : (c + 1) * R]),
            "rhs_aug": rhs_aug,
            "w_src_rep": w_src_rep,
            "src_bias": src_bias,
            "s_rep": s_rep,
            "bdst": bdst,
            "nbdst": -bdst,
        }
        if with_fcb:
            m["fcbz"] = np.tile(
                np.concatenate([fc_b, [np.float32(0)]]).astype(np.float32)[None, :],
                (128, 1),
            )
        in_maps.append(m)
    return with_fcb, in_maps


def kernel(adj, x, fc_w, fc_b, attn_w, attn_b, _trace=False, _tmpdir=None):
    from concourse import bass_utils

    with_fcb, in_maps = _prep_inputs(adj, x, fc_w, fc_b, attn_w, attn_b)
    key = ("nc", with_fcb)
    if key not in _cache:
        _cache[key] = _build(with_fcb)
    nc = _cache[key]
    res = bass_utils.run_bass_kernel_spmd(
        nc,
        in_maps,
        core_ids=list(range(NCORES)),
        trace=_trace,
        **({"tmpdir": _tmpdir} if _tmpdir else {}),
    )
    out = np.concatenate([res.results[c]["out"] for c in range(NCORES)], axis=0)
    if _trace:
        _cache["last_exec_time_ns"] = res.exec_time_ns
        _cache["last_profile_json"] = res.profile_json
    return out
